# revision 10
# baseline (speedup 1.0000x reference)
"""Trainium2 Bass kernel for nn_DwTNL_module (sparse attention).

Sharding: data-parallel over batch b — 8 batches onto 8 NeuronCores,
weights/BN params replicated. Each core computes its batch end-to-end
(no collectives).

Per-core pipeline (b fixed, cr=128 on partitions):
  phase 1 (per t): x_t -> Q/K/V 1x1 convs (PE matmuls, K=256 split in 2)
                   -> exact sorted top-196 of Q,K rows via max8/match_replace
                   -> PE-transpose top-k tiles into [k, t*cr] layout
  phase 2 (per group of 8 cr-rows): corr = Qtk @ Ktk^T batched 8-rows/matmul
                   -> diag extract (mask-mult + strided reduce) -> softmax
                   -> block-diag attention matrix (broadcast-mult + PE transpose)
                   -> y = BD^T @ V_unfolded  -> fold y back to [cr, t*hw] via DMA
  phase 3 (per 512-col chunk): recon 1x1 conv (PE) + folded BN affine (ACT)
                   + residual add (DVE) -> DMA out
"""
import numpy as np
from contextlib import ExitStack

import concourse.bass as bass
import concourse.tile as tile
from concourse import bacc, mybir
from concourse.masks import make_identity

f32 = mybir.dt.float32

B, C, T, H, W = 8, 256, 16, 28, 28
CR = 128
HWD = H * W            # 784
TH = T * HWD           # 12544
KTOP = 196
NROUND = 25            # ceil(196/8)
KPAD = NROUND * 8      # 200
NEG = -1e30
NGROUP = 16            # 128 cr rows / 8 per group
RCHUNK = 512
NRCHUNK = (TH + RCHUNK - 1) // RCHUNK  # 25 (24x512 + 256)


def build_nc():
    nc = bacc.Bacc("TRN2", target_bir_lowering=False, debug=False)

    x_d = nc.declare_dram_parameter("xb", [C, TH], f32, isOutput=False)
    wq_d = nc.declare_dram_parameter("wq2", [CR, C], f32, isOutput=False)
    wk_d = nc.declare_dram_parameter("wk2", [CR, C], f32, isOutput=False)
    wv_d = nc.declare_dram_parameter("wv2", [CR, C], f32, isOutput=False)
    wr_d = nc.declare_dram_parameter("wr2", [CR, C], f32, isOutput=False)
    b3_d = nc.declare_dram_parameter("b3", [CR, 3], f32, isOutput=False)
    bn4_d = nc.declare_dram_parameter("bn4", [CR, 4], f32, isOutput=False)
    msk_d = nc.declare_dram_parameter("mask", [128, 128], f32, isOutput=False)
    out_d = nc.declare_dram_parameter("out", [C, TH], f32, isOutput=True)

    with tile.TileContext(nc) as tc, ExitStack() as ctx:
        cpool = ctx.enter_context(tc.tile_pool(name="consts", bufs=1))

        # constants — each tile written by exactly ONE DMA (LDW wait-slot limit)
        wq = cpool.tile([128, 2 * CR], f32)   # [k-chunk0 cols | k-chunk1 cols]
        wk = cpool.tile([128, 2 * CR], f32)
        wv = cpool.tile([128, 2 * CR], f32)
        wr = cpool.tile([128, C], f32)
        nc.sync.dma_start(wq[:], wq_d.ap())
        nc.sync.dma_start(wk[:], wk_d.ap())
        nc.sync.dma_start(wv[:], wv_d.ap())
        nc.sync.dma_start(wr[:], wr_d.ap())
        b3 = cpool.tile([128, 3], f32)
        nc.sync.dma_start(b3[:], b3_d.ap())
        bq, bk, bv = b3[:, 0:1], b3[:, 1:2], b3[:, 2:3]
        bn4 = cpool.tile([128, 4], f32)
        nc.sync.dma_start(bn4[:], bn4_d.ap())
        osc, osh = bn4[:, 0:2], bn4[:, 2:4]
        bdmask = cpool.tile([128, 128], f32)
        nc.sync.dma_start(bdmask[:], msk_d.ap())
        ident = cpool.tile([128, 128], f32)
        make_identity(nc, ident[:])

        # long-lived tensors
        big = ctx.enter_context(tc.tile_pool(name="big", bufs=1))
        v_full = big.tile([128, TH], f32)
        y_full = big.tile([128, TH], f32)
        q1T = big.tile([128, T * 128], f32)   # Qtk^T k=0..128
        q2T = big.tile([128, T * 128], f32)   # Qtk^T k=128..196 (68 partitions used)
        k1T = big.tile([128, T * 128], f32)
        k2T = big.tile([128, T * 128], f32)

        # ---------------- phase 1 ----------------
        with tc.tile_pool(name="xt", bufs=3) as xt_pool, \
             tc.tile_pool(name="proj_ps", bufs=2, space="PSUM") as proj_ps, \
             tc.tile_pool(name="qk_sb", bufs=3) as qk_pool, \
             tc.tile_pool(name="qtk", bufs=3) as qtk_pool, \
             tc.tile_pool(name="tp_ps", bufs=2, space="PSUM") as tp_ps:

            for t in range(T):
                sl = slice(t * HWD, (t + 1) * HWD)
                xa = xt_pool.tile([128, HWD], f32, tag="xa")
                xb2 = xt_pool.tile([128, HWD], f32, tag="xb")
                nc.sync.dma_start(xa[:], x_d.ap()[0:128, sl])
                nc.sync.dma_start(xb2[:], x_d.ap()[128:256, sl])

                def proj(wt, bias, out_sb):
                    ps = proj_ps.tile([128, HWD], f32, tag="proj")
                    for n0, n1 in ((0, 512), (512, HWD)):
                        nc.tensor.matmul(ps[:, n0:n1], wt[:, 0:CR], xa[:, n0:n1],
                                         start=True, stop=False)
                        nc.tensor.matmul(ps[:, n0:n1], wt[:, CR:2 * CR], xb2[:, n0:n1],
                                         start=False, stop=True)
                    nc.scalar.activation(out_sb, ps[:],
                                         mybir.ActivationFunctionType.Identity,
                                         bias=bias[:], scale=1.0)

                qsb = qk_pool.tile([128, HWD], f32, tag="q")
                ksb = qk_pool.tile([128, HWD], f32, tag="k")
                proj(wq, bq, qsb[:])
                proj(wk, bk, ksb[:])
                proj(wv, bv, v_full[:, sl])

                for src, t1, t2 in ((qsb, q1T, q2T), (ksb, k1T, k2T)):
                    tk = qtk_pool.tile([128, KPAD], f32, tag="tk")
                    for r in range(NROUND):
                        nc.vector.max(out=tk[:, 8 * r:8 * r + 8], in_=src[:])
                        if r < NROUND - 1:
                            nc.vector.match_replace(out=src[:], in_to_replace=tk[:, 8 * r:8 * r + 8],
                                                    in_values=src[:], imm_value=NEG)
                    # store transposed topk in (cr-major, t-minor) cols: col = c*16 + t
                    # so the per-group matmul operand is one contiguous 128-col slice
                    t1v = t1[:].rearrange("p (c tt) -> p tt c", tt=T)
                    t2v = t2[:].rearrange("p (c tt) -> p tt c", tt=T)
                    ps1 = tp_ps.tile([128, 128], f32, tag="tp")
                    nc.tensor.transpose(ps1[:], tk[:, 0:128], ident[:])
                    nc.scalar.copy(t1v[:, t, :], ps1[:])
                    ps2 = tp_ps.tile([128, 128], f32, tag="tp")
                    nc.tensor.transpose(ps2[0:68, :], tk[:, 128:KTOP], ident[:])
                    nc.scalar.copy(t2v[0:68, t, :], ps2[0:68, :])

        # ---------------- phase 2 ----------------
        def gap(tile_, i, np_=128):
            # cols are (c-major, t-minor): group i = contiguous cols [128i, 128i+128)
            return tile_[:np_, 128 * i:128 * (i + 1)]

        with tc.tile_pool(name="attn_ps", bufs=2, space="PSUM") as attn_ps, \
             tc.tile_pool(name="y_ps", bufs=2, space="PSUM") as y_ps, \
             tc.tile_pool(name="attn_sb", bufs=2) as attn_sb, \
             tc.tile_pool(name="vunf", bufs=2) as vunf_pool, \
             tc.tile_pool(name="ysb", bufs=2) as ysb_pool:

            for i in range(NGROUP):
                ps_corr = attn_ps.tile([128, 128], f32, tag="corr")
                nc.tensor.matmul(ps_corr[:], gap(q1T, i), gap(k1T, i), start=True, stop=False)
                nc.tensor.matmul(ps_corr[:], gap(q2T, i, 68), gap(k2T, i, 68), start=False, stop=True)

                corr_m = attn_sb.tile([128, 128], f32, tag="corrm")
                nc.vector.tensor_tensor(out=corr_m[:], in0=ps_corr[:], in1=bdmask[:],
                                        op=mybir.AluOpType.mult)
                lg = attn_sb.tile([128, 16], f32, tag="lg")
                nc.vector.tensor_reduce(out=lg[:].rearrange("p (s o) -> p s o", o=1),
                                        in_=corr_m[:].rearrange("p (jp s) -> p s jp", jp=8),
                                        op=mybir.AluOpType.add, axis=mybir.AxisListType.X)

                mx = attn_sb.tile([128, 1], f32, tag="mx")
                nc.vector.tensor_reduce(out=mx[:], in_=lg[:], op=mybir.AluOpType.max,
                                        axis=mybir.AxisListType.X)
                nmx = attn_sb.tile([128, 1], f32, tag="nmx")
                nc.vector.tensor_scalar_mul(nmx[:], mx[:], -1.0)
                ssum = attn_sb.tile([128, 1], f32, tag="ssum")
                nc.scalar.activation(lg[:], lg[:], mybir.ActivationFunctionType.Exp,
                                     bias=nmx[:], scale=1.0, accum_out=ssum[:])
                rsum = attn_sb.tile([128, 1], f32, tag="rsum")
                nc.vector.reciprocal(rsum[:], ssum[:])
                nc.vector.tensor_scalar_mul(lg[:], lg[:], rsum[:])

                tt = attn_sb.tile([128, 128], f32, tag="tt")
                nc.vector.tensor_tensor(
                    out=tt[:].rearrange("p (jp s) -> p jp s", jp=8),
                    in0=lg[:].rearrange("p (jp s) -> p jp s", jp=1).to_broadcast([128, 8, 16]),
                    in1=bdmask[:].rearrange("p (jp s) -> p jp s", jp=8),
                    op=mybir.AluOpType.mult)
                ps_bd = attn_ps.tile([128, 128], f32, tag="bd")
                nc.tensor.transpose(ps_bd[:], tt[:], ident[:])
                bd = attn_sb.tile([128, 128], f32, tag="bdsb")
                nc.scalar.copy(bd[:], ps_bd[:])

                vunf = vunf_pool.tile([128, HWD], f32, tag="vunf")
                for j in range(8):
                    nc.sync.dma_start(vunf[16 * j:16 * j + 16, :],
                                      v_full[8 * i + j:8 * i + j + 1, :])

                ps_y = y_ps.tile([128, HWD], f32, tag="y")
                nc.tensor.matmul(ps_y[:, 0:512], bd[:], vunf[:, 0:512], start=True, stop=True)
                nc.tensor.matmul(ps_y[:, 512:HWD], bd[:], vunf[:, 512:HWD], start=True, stop=True)
                ysb = ysb_pool.tile([128, HWD], f32, tag="ysb")
                nc.scalar.copy(ysb[:], ps_y[:])
                for j in range(8):
                    nc.sync.dma_start(y_full[8 * i + j:8 * i + j + 1, :],
                                      ysb[16 * j:16 * j + 16, :])

        # ---------------- phase 3 ----------------
        with tc.tile_pool(name="rec_ps", bufs=4, space="PSUM") as rec_ps, \
             tc.tile_pool(name="rec_sb", bufs=4) as rec_sb, \
             tc.tile_pool(name="xres", bufs=4) as xres_pool:

            for ci in range(NRCHUNK):
                n0 = ci * RCHUNK
                n1 = min(TH, n0 + RCHUNK)
                w_ = n1 - n0
                for oc in range(2):
                    ps = rec_ps.tile([128, RCHUNK], f32, tag="rec")
                    nc.tensor.matmul(ps[:, 0:w_], wr[:, 128 * oc:128 * oc + 128],
                                     y_full[:, n0:n1], start=True, stop=True)
                    ob = rec_sb.tile([128, RCHUNK], f32, tag="ob")
                    nc.scalar.activation(ob[:, 0:w_], ps[:, 0:w_],
                                         mybir.ActivationFunctionType.Identity,
                                         bias=osh[:, oc:oc + 1], scale=osc[:, oc:oc + 1])
                    xr = xres_pool.tile([128, RCHUNK], f32, tag="xr")
                    nc.sync.dma_start(xr[:, 0:w_], x_d.ap()[128 * oc:128 * oc + 128, n0:n1])
                    nc.vector.tensor_add(ob[:, 0:w_], ob[:, 0:w_], xr[:, 0:w_])
                    nc.sync.dma_start(out_d.ap()[128 * oc:128 * oc + 128, n0:n1], ob[:, 0:w_])

    nc.compile()
    return nc


def _host_prep(inputs):
    x = np.ascontiguousarray(np.asarray(inputs['x'], dtype=np.float32))
    Wq = np.asarray(inputs['Wq'], dtype=np.float32)
    Wk = np.asarray(inputs['Wk'], dtype=np.float32)
    Wv = np.asarray(inputs['Wv'], dtype=np.float32)
    Wr = np.asarray(inputs['Wr'], dtype=np.float32)
    gamma = np.asarray(inputs['gamma'], dtype=np.float32)
    beta = np.asarray(inputs['beta'], dtype=np.float32)
    mean = np.asarray(inputs['bn_mean'], dtype=np.float32)
    var = np.asarray(inputs['bn_var'], dtype=np.float32)
    br = np.asarray(inputs['br'], dtype=np.float32)

    scale = gamma / np.sqrt(var + 1e-5)
    oshift = br * scale + beta - mean * scale

    def pack_w(wT):  # [256,128] -> [128, 256] = [rows 0:128 | rows 128:256]
        return np.ascontiguousarray(np.concatenate([wT[:128, :], wT[128:, :]], axis=1))

    common = {
        'wq2': pack_w(Wq.T),
        'wk2': pack_w(Wk.T),
        'wv2': pack_w(Wv.T),
        'wr2': np.ascontiguousarray(Wr.T),
        'b3': np.ascontiguousarray(np.stack([
            np.asarray(inputs['bq'], dtype=np.float32),
            np.asarray(inputs['bk'], dtype=np.float32),
            np.asarray(inputs['bv'], dtype=np.float32)], axis=1)),
        'bn4': np.ascontiguousarray(np.stack([
            scale[:128], scale[128:], oshift[:128], oshift[128:]], axis=1)).astype(np.float32),
        'mask': _bdmask(),
    }
    in_maps = []
    for b in range(B):
        m = dict(common)
        m['xb'] = np.ascontiguousarray(x[b].reshape(C, TH))
        in_maps.append(m)
    return in_maps


def _bdmask():
    m = np.zeros((128, 128), dtype=np.float32)
    for j in range(8):
        m[16 * j:16 * j + 16, 16 * j:16 * j + 16] = 1.0
    return m


def kernel(**inputs):
    from concourse.bass_utils import run_bass_kernel_spmd
    nc = build_nc()
    in_maps = _host_prep(inputs)
    res = run_bass_kernel_spmd(nc, in_maps, list(range(B)))
    out = np.stack([np.asarray(res.results[b]['out']) for b in range(B)], axis=0)
    return out.reshape(B, C, T, H, W).astype(np.float32)
